# revision 31
# baseline (speedup 1.0000x reference)
"""Multi-head causal attention on 8 TRN2 NeuronCores.

Sharding: core c -> batch c//2, head-group c%2 (8 of 16 heads).
Wq/Wk/Wv column-sharded, Wo row-sharded; the Wo all-reduce is the host-side
sum of the two partial outputs per batch.

Per-core kernel (Bass/Tile):
  phase A: kT = Wk^T x_k^T [512, 2048] (transposed), v = x_v Wv [2048, 8, 65]
           (65th column per head = ones -> softmax denominator via PV matmul)
  per sq-tile t (512 queries):
    qT_t = (Wq*s)^T x_q^T slice [512, 512]
    head pairs (PE row groups 0-63/64-127 run QK concurrently):
      attnT chunks [sk 128, sq 512] = kT^T qT, exp on ACT, causal zeroing via
      gpsimd affine_select (identity-matmul additive mask for general masks),
      PV accumulates out^T[65, 512] (row 64 = denominator).
    epilogue (deferred one pair): reciprocal -> PE broadcast -> normalize,
    DMA into aoT. Output projection of tile t-1 interleaved into tile t's
    attention to fill PE gaps.
"""

import os
import sys

for _p in ("/opt/trn_rl_repo", "/root/.axon_site/_ro/trn_rl_repo"):
    if os.path.isdir(_p) and _p not in sys.path:
        sys.path.insert(0, _p)

import numpy as np

import concourse.bass as bass  # noqa: F401
import concourse.tile as tile
from concourse import bacc, mybir
from concourse.bass_utils import run_bass_kernel_spmd
from concourse.dve_ops import RECIP_APPROX_FAST_CONSTS as _RC
from concourse.dve_ops import RECIPROCAL_APPROX_FAST
import ml_dtypes

BF16_NP = ml_dtypes.bfloat16

F32 = mybir.dt.float32
F32R = mybir.dt.float32r
BF16 = mybir.dt.bfloat16

B, S, D = 4, 2048, 1024
H, DH = 16, 64
SCALE = DH**-0.5
NCORES = 8
NHPC = 8
HDPC = NHPC * DH  # 512
SQT = 512
NSQT = S // SQT  # 4
SKC = 128
NSKC = S // SKC  # 16
NDC = D // 128  # 8
NMC = HDPC // 128  # 4
NEG = -1.0e30

CFG = {
    "qk": "bf16",
    "pv": "bf16",
    "ao": "bf16",
    "xbufs": 4,
    "qtbufs": 2,
    "ebufs": 5,
    "sbufs": 2,
    "psq": 4,
    "psv": 4,
}

LAST_RESULTS = None

_DT = {"f32r": F32R, "bf16": BF16, "fp16": mybir.dt.float16}


def _mask_layout(mask: np.ndarray):
    """Blocks of [sk=128, sq=512].  Returns chunks[t] = list of
    (c, kind, arg): kind 'clear' (no masking), 'affine' (causal-style
    triangle, arg = affine base), or 'madd' (arg = packed additive tile idx).
    Fully-masked blocks are dropped."""
    chunks = []
    uniq = {}
    madds = []
    rr = np.arange(SKC)[:, None]
    jj = np.arange(SQT)[None, :]
    for t in range(NSQT):
        lst = []
        for c in range(NSKC):
            sub = mask[t * SQT : (t + 1) * SQT, c * SKC : (c + 1) * SKC]
            if sub.all():
                continue
            if not sub.any():
                lst.append((c, "clear", 0))
                continue
            subT = sub.T
            base = c * SKC - t * SQT
            if np.array_equal(subT, (rr + base) > jj):
                lst.append((c, "affine", base))
                continue
            key = subT.tobytes()
            if key not in uniq:
                madds.append(np.where(subT, NEG, 0.0).astype(np.float32))
                uniq[key] = len(madds) - 1
            lst.append((c, "madd", uniq[key]))
        assert lst, f"sq tile {t} fully masked"
        chunks.append(lst)
    madd_arr = (
        np.stack(madds) if madds else np.zeros((1, SKC, SQT), dtype=np.float32)
    )
    return chunks, madd_arr, bool(madds)


def _build_program(chunks, n_madd, use_madd, cfg, tick=False, reps=1):
    qk_dt = _DT[cfg["qk"]]
    pv_dt = _DT[cfg["pv"]]
    ao_dt = _DT[cfg["ao"]]

    nc = bacc.Bacc(
        "TRN2", target_bir_lowering=False, debug=False, num_devices=NCORES
    )
    if tick:
        tick_ap = nc.dram_tensor("tick", [128, 8], F32, kind="ExternalInput").ap()
    xqT = nc.dram_tensor("xqT", [D, S], BF16, kind="ExternalInput").ap()
    xkT = nc.dram_tensor("xkT", [D, S], BF16, kind="ExternalInput").ap()
    xvT = nc.dram_tensor("xvT", [D, S], BF16, kind="ExternalInput").ap()
    wq = nc.dram_tensor("wq", [D, HDPC], BF16, kind="ExternalInput").ap()
    wk = nc.dram_tensor("wk", [D, HDPC], BF16, kind="ExternalInput").ap()
    wv = nc.dram_tensor("wv", [D, HDPC], BF16, kind="ExternalInput").ap()
    wo = nc.dram_tensor("wo", [HDPC, D], BF16, kind="ExternalInput").ap()
    if use_madd:
        madd = nc.dram_tensor(
            "madd", [n_madd, SKC, SQT], BF16, kind="ExternalInput"
        ).ap()
    out = nc.dram_tensor("out", [S, D], F32, kind="ExternalOutput").ap()

    with tile.TileContext(nc) as tc:
        with (
            tc.tile_pool(name="const", bufs=1) as const,
            tc.tile_pool(name="wbig", bufs=3) as wbig,
            tc.tile_pool(name="xpool", bufs=cfg["xbufs"]) as xpool,
            tc.tile_pool(name="qtp", bufs=cfg["qtbufs"]) as qtp,
            tc.tile_pool(name="aop", bufs=2) as aop,
            tc.tile_pool(name="big", bufs=1) as big,
            tc.tile_pool(name="epool", bufs=cfg["ebufs"]) as epool,
            tc.tile_pool(name="spool", bufs=cfg["sbufs"]) as spool,
            tc.tile_pool(name="opool", bufs=2) as opool,
            tc.tile_pool(name="psq", bufs=cfg["psq"], space="PSUM") as psq,
            tc.tile_pool(name="psv", bufs=cfg["psv"], space="PSUM") as psv,
        ):
            # ---- constants ----
            if tick:
                tick_sb = const.tile([128, 8], F32)
                nc.sync.dma_start(tick_sb, tick_ap)
            ones_plane = const.tile([128, 128], F32)
            nc.vector.memset(ones_plane, 1.0)
            if use_madd:
                ident_sb = const.tile([128, 128], BF16)
                nc.gpsimd.memset(ident_sb, 0.0)
                nc.gpsimd.affine_select(
                    out=ident_sb,
                    in_=ident_sb,
                    compare_op=mybir.AluOpType.not_equal,
                    fill=1.0,
                    base=0,
                    pattern=[[-1, 128]],
                    channel_multiplier=1,
                )
                madd_sb = const.tile([SKC, n_madd, SQT], BF16)
                nc.gpsimd.dma_start(madd_sb, madd.rearrange("n p s -> p n s"))
            wo_sb = const.tile([128, NMC, D], ao_dt)
            nc.sync.dma_start(wo_sb, wo.rearrange("(c p) m -> p c m", p=128))

            def emit_body():
                # ---- persistent tiles (split per sq-tile n so attention for
                # tile t only depends on kT/v pieces n <= t, letting the
                # scheduler overlap the K/V projections with attention) ----
                kT_n = [
                    big.tile(
                        [128, NMC, SQT], qk_dt, tag=f"kT{n}", name=f"kT{n}"
                    )
                    for n in range(NSQT)
                ]
                v_n = [
                    big.tile(
                        [128, 4, NHPC, DH + 1], pv_dt, tag=f"v{n}", name=f"v{n}"
                    )
                    for n in range(NSQT)
                ]
                for n in range(NSQT):
                    nc.vector.tensor_copy(
                        v_n[n][:, :, :, DH : DH + 1],
                        ones_plane.rearrange(
                            "p (a b c) -> p a b c", a=4, b=NHPC
                        )[:, :, :, 0:1],
                    )

                def load_xT(src, n):
                    xt = xpool.tile([128, NDC, SQT], BF16, tag="xt")
                    nc.sync.dma_start(
                        xt,
                        src[:, n * SQT : (n + 1) * SQT].rearrange(
                            "(c p) s -> p c s", p=128
                        ),
                    )
                    return xt

                def load_w(wsrc):
                    wt = wbig.tile([128, NDC, HDPC], BF16, tag="wt")
                    nc.sync.dma_start(
                        wt, wsrc.rearrange("(c p) m -> p c m", p=128)
                    )
                    return wt

                # ---- weights up front (wbig holds wk+wv+wq) ----
                wtk = load_w(wk)
                wtv = load_w(wv)
                wtq = load_w(wq)

                def emit_kv_piece(n):
                    xtk = load_xT(xkT, n)
                    for m in range(NMC):
                        ps = psq.tile([128, SQT], F32, tag="ps512")
                        for kc in range(NDC):
                            nc.tensor.matmul(
                                ps,
                                wtk[:, kc, m * 128 : (m + 1) * 128],
                                xtk[:, kc, :],
                                start=(kc == 0),
                                stop=(kc == NDC - 1),
                            )
                        nc.vector.tensor_copy(kT_n[n][:, m, :], ps)
                    xtv = load_xT(xvT, n)
                    for si in range(4):
                        ps = psq.tile([128, SQT], F32, tag="ps512")
                        for kc in range(NDC):
                            nc.tensor.matmul(
                                ps,
                                xtv[:, kc, si * 128 : (si + 1) * 128],
                                wtv[:, kc, :],
                                start=(kc == 0),
                                stop=(kc == NDC - 1),
                            )
                        nc.vector.tensor_copy(
                            v_n[n][:, si, :, 0:DH],
                            ps.rearrange("p (h e) -> p h e", h=NHPC),
                        )

                # ---- deferred work helpers ----
                def epilogue(pv, h, aoT_t):
                    mc_ = h // 2
                    ro = (h % 2) * 64
                    # custom-DVE ops can't read PSUM: bounce the denom row to
                    # SBUF, then ~51-ULP reciprocal (single DVE pass), then
                    # GPSIMD partition-broadcast (keeps PE out of the epilogue)
                    dscr = spool.tile([1, SQT], F32, tag="dscr")
                    nc.vector.tensor_copy(dscr, pv[64:65, :])
                    drec = spool.tile([1, SQT], F32, tag="drec")
                    nc.vector._custom_dve(
                        RECIPROCAL_APPROX_FAST,
                        out=drec,
                        in0=dscr,
                        s0=_RC["s0"],
                        s1=_RC["s1"],
                        imm2=_RC["imm2"],
                    )
                    bcast = spool.tile([64, SQT], F32, tag="bcast")
                    nc.gpsimd.partition_broadcast(bcast, drec)
                    tmp = spool.tile([64, SQT], ao_dt, tag="tmp")
                    nc.vector.tensor_mul(tmp, pv[0:64, :], bcast)
                    nc.sync.dma_start(aoT_t[ro : ro + 64, mc_, :], tmp)

                def outproj_sc(aoT_prev, sc):
                    si = sc % 4
                    for j in range(2):
                        po = psq.tile([128, 512], F32, tag="ps512")
                        for mc2 in range(NMC):
                            nc.tensor.matmul(
                                po,
                                aoT_prev[:, mc2, si * 128 : (si + 1) * 128],
                                wo_sb[:, mc2, j * 512 : (j + 1) * 512],
                                start=(mc2 == 0),
                                stop=(mc2 == NMC - 1),
                            )
                        o_sb = opool.tile([128, 512], F32, tag="o")
                        nc.vector.tensor_copy(o_sb, po)
                        nc.sync.dma_start(
                            out[sc * 128 : (sc + 1) * 128, j * 512 : (j + 1) * 512],
                            o_sb,
                        )

                # ---- per sq-tile: qT_t, attention (head pairs), outproj(t-1) ----
                prev_epi = None  # (pvA, pvB, hA, hB, aoT_t)
                aoT_prev = None
                for t in range(NSQT):
                    emit_kv_piece(t)
                    xt = load_xT(xqT, t)
                    qT_t = qtp.tile([128, NMC, SQT], qk_dt, tag="qT")
                    for m in range(NMC):
                        ps = psq.tile([128, SQT], F32, tag="ps512")
                        for kc in range(NDC):
                            nc.tensor.matmul(
                                ps,
                                wtq[:, kc, m * 128 : (m + 1) * 128],
                                xt[:, kc, :],
                                start=(kc == 0),
                                stop=(kc == NDC - 1),
                            )
                        nc.vector.tensor_copy(qT_t[:, m, :], ps)
                        if m == 0 and prev_epi is not None:
                            pvA, pvB, hA, hB, ao_ = prev_epi
                            epilogue(pvA, hA, ao_)
                            epilogue(pvB, hB, ao_)
                            prev_epi = None

                    aoT_t = aop.tile([128, NMC, SQT], ao_dt, tag="aoT")
                    for hp in range(NHPC // 2):
                        hA, hB = 2 * hp, 2 * hp + 1
                        qsA = qT_t[0:64, hp, :]
                        qsB = qT_t[64:128, hp, :]
                        pvA = psv.tile([65, SQT], F32, tag="pv")
                        pvB = psv.tile([65, SQT], F32, tag="pv")
                        pend = None
                        for ci, (c, kind, arg) in enumerate(chunks[t]):
                            # first unmasked sq column of this chunk: columns
                            # left of it are fully masked (causal diag blocks)
                            # and never computed — affine_select zeroes them.
                            sl = arg if kind == "affine" else 0
                            qkA = psq.tile([128, SQT], F32, tag="ps512")
                            qkB = psq.tile([128, SQT], F32, tag="ps512")
                            last = kind != "madd"
                            kT_c = kT_n[c // 4]
                            co = (c % 4) * SKC
                            nc.tensor.matmul(
                                qkA[:, sl:],
                                kT_c[0:64, hp, co : co + SKC],
                                qsA[:, sl:],
                                start=True,
                                stop=last,
                            )
                            nc.tensor.matmul(
                                qkB[:, sl:],
                                kT_c[64:128, hp, co : co + SKC],
                                qsB[:, sl:],
                                start=True,
                                stop=last,
                            )
                            if kind == "madd":
                                nc.tensor.matmul(
                                    qkA, ident_sb, madd_sb[:, arg, :],
                                    start=False, stop=True,
                                )
                                nc.tensor.matmul(
                                    qkB, ident_sb, madd_sb[:, arg, :],
                                    start=False, stop=True,
                                )
                            eA = epool.tile([SKC, SQT], pv_dt, tag="e")
                            eB = epool.tile([SKC, SQT], pv_dt, tag="e")
                            nc.scalar.activation(
                                eA[:, sl:], qkA[:, sl:],
                                mybir.ActivationFunctionType.Exp,
                            )
                            nc.scalar.activation(
                                eB[:, sl:], qkB[:, sl:],
                                mybir.ActivationFunctionType.Exp,
                            )
                            if kind == "affine":
                                # keep e[r, j] iff j - r - base >= 0 (i.e.
                                # sk <= sq); also zeroes the uncomputed
                                # columns j < base
                                for e_ in (eA, eB):
                                    nc.gpsimd.affine_select(
                                        out=e_,
                                        in_=e_,
                                        compare_op=mybir.AluOpType.is_ge,
                                        fill=0.0,
                                        base=-arg,
                                        pattern=[[1, SQT]],
                                        channel_multiplier=-1,
                                    )
                            if pend is not None:
                                pc, peA, peB, pci, psl = pend
                                nc.tensor.matmul(
                                    pvA[:, psl:], v_n[pc // 4][:, pc % 4, hA, :], peA[:, psl:],
                                    start=(pci == 0), stop=False,
                                )
                                nc.tensor.matmul(
                                    pvB[:, psl:], v_n[pc // 4][:, pc % 4, hB, :], peB[:, psl:],
                                    start=(pci == 0), stop=False,
                                )
                            pend = (c, eA, eB, ci, sl)
                            if ci == 1 and prev_epi is not None:
                                ppA, ppB, phA, phB, ao_ = prev_epi
                                epilogue(ppA, phA, ao_)
                                epilogue(ppB, phB, ao_)
                                prev_epi = None
                        pc, peA, peB, pci, psl = pend
                        nc.tensor.matmul(
                            pvA[:, psl:], v_n[pc // 4][:, pc % 4, hA, :], peA[:, psl:],
                            start=(pci == 0), stop=True,
                        )
                        nc.tensor.matmul(
                            pvB[:, psl:], v_n[pc // 4][:, pc % 4, hB, :], peB[:, psl:],
                            start=(pci == 0), stop=True,
                        )
                        prev_epi = (pvA, pvB, hA, hB, aoT_t)
                        if aoT_prev is not None:
                            outproj_sc(aoT_prev, (t - 1) * 4 + hp)
                    aoT_prev = aoT_t

                # flush: last pair epilogue + last tile's output projection
                pvA, pvB, hA, hB, ao_ = prev_epi
                epilogue(pvA, hA, ao_)
                epilogue(pvB, hB, ao_)
                for si in range(4):
                    outproj_sc(aoT_prev, (NSQT - 1) * 4 + si)

            for _rep in range(reps):
                emit_body()

    nc.finalize()
    return nc


_PROG_CACHE = {}


def kernel(x_q, x_k, x_v, mask, Wq, Wk, Wv, Wo):
    global LAST_RESULTS
    x_q = np.asarray(x_q, dtype=np.float32)
    x_k = np.asarray(x_k, dtype=np.float32)
    x_v = np.asarray(x_v, dtype=np.float32)
    mask = np.asarray(mask).astype(bool)
    Wq = np.asarray(Wq, dtype=np.float32)
    Wk = np.asarray(Wk, dtype=np.float32)
    Wv = np.asarray(Wv, dtype=np.float32)
    Wo = np.asarray(Wo, dtype=np.float32)

    chunks, madd_arr, use_madd = _mask_layout(mask)
    key = (
        tuple(tuple(lst) for lst in chunks),
        madd_arr.shape[0],
        use_madd,
        tuple(sorted(CFG.items())),
    )
    if key not in _PROG_CACHE:
        _PROG_CACHE[key] = _build_program(
            chunks, madd_arr.shape[0], use_madd, CFG
        )
    nc = _PROG_CACHE[key]

    wq_s = np.ascontiguousarray(Wq * np.float32(SCALE))
    xqT_b = [np.ascontiguousarray(x_q[b].T.astype(BF16_NP)) for b in range(B)]
    xkT_b = [np.ascontiguousarray(x_k[b].T.astype(BF16_NP)) for b in range(B)]
    xvT_b = [np.ascontiguousarray(x_v[b].T.astype(BF16_NP)) for b in range(B)]
    madd_bf = madd_arr.astype(BF16_NP)
    in_maps = []
    for c in range(NCORES):
        b = c // 2
        hs = slice((c % 2) * HDPC, (c % 2 + 1) * HDPC)
        m = {
            "xqT": xqT_b[b],
            "xkT": xkT_b[b],
            "xvT": xvT_b[b],
            "wq": np.ascontiguousarray(wq_s[:, hs].astype(BF16_NP)),
            "wk": np.ascontiguousarray(Wk[:, hs].astype(BF16_NP)),
            "wv": np.ascontiguousarray(Wv[:, hs].astype(BF16_NP)),
            "wo": np.ascontiguousarray(Wo[hs, :].astype(BF16_NP)),
        }
        if use_madd:
            m["madd"] = madd_bf
        in_maps.append(m)

    res = run_bass_kernel_spmd(nc, in_maps, core_ids=list(range(NCORES)))
    LAST_RESULTS = res
    out = np.empty((B, S, D), dtype=np.float32)
    for b in range(B):
        out[b] = res.results[2 * b]["out"] + res.results[2 * b + 1]["out"]
    return out

